# revision 19
# baseline (speedup 1.0000x reference)
"""ARMA(4,4) generator as a truncated-impulse-response convolution on TRN2.

Math: the reference recurrence
    x_t = mu + sum_i phi_i x_{t-i} + e_t + sum_j theta_j e_{t-j}
is a linear time-invariant filter applied to u_t = e_t + c_t (where c solves
c_t + sum_j theta_j c_{t-j} = mu, making the mu term exact), independently per
(sequence, channel):
    x[n, t, d] = sum_k g_d[k] * u[n, t-k, d]   (+ response to x0, zero here)
with g_d the ARMA impulse response (g[0] = 1).  g decays geometrically
(stationary filter); each channel's 128-tap block count NK_d is chosen at
runtime from the actual phi/theta so the truncation error stays below 3e-5.

Device kernel: per channel d, the causal convolution over a 128-step time
block is a lower-triangular block-Toeplitz matmul — time on SBUF partitions,
(sequence, block) pairs as matmul columns.  No serial recurrence remains.

Performance: the kernel is DMA-bound (all HBM transfers serialize at
~360 GB/s per core), so every byte on the wire is critical path:
- fp16 end-to-end: input u, Toeplitz weights and the output all move at
  2 B/elem; fp32 PSUM accumulation keeps the conv itself exact.  Total
  quantization loss ~3e-4 rel err vs the fp32 reference (gate is 2e-2).
- Weights are loaded once and stay SBUF-resident.  The j>=1 blocks are
  nonzero only in their bottom-left K_d x K_d corner (K_d = per-channel
  tap horizon, typically 15-80), so they ship column-trimmed to K_d cols
  and row-trimmed to the bottom 64 partitions where legal (engine APs
  allow base partition 0 or 64 only): 4.3 MB dense -> ~2.5 MB.
- Schedule: weights + all four input-chunk DMAs are issued up front (SP's
  SEQ is in-order, so input loads emitted after output DMAs would stall
  behind their sem waits and starve the DMA engines); 8 output buffers
  cover two full chunks of production so compute never blocks on SBUF
  while the input stream monopolizes the DMA engines.  Result: the DMA
  stream runs gapless from first to last transfer.
Per-core traffic 16.75 MB in + 2.5 MB weights + 16.75 MB out -> ~104 us,
vs 99 MB / ~281 us for the fp32+hi/lo baseline (2.7x).  PE (~56 us) and
the PSUM->SBUF evacuation copies (~50 us/engine on Act+DVE) hide under
the DMA stream.

Sharding: pure data parallelism — 32 of the 256 sequences per NeuronCore.
"""

import os
import numpy as np

N, T, D, P, Q = 256, 4096, 64, 4, 4
NCORES = 8
SEQ_PER_CORE = N // NCORES          # 32
BLK = 128                           # time block = SBUF partition count
TB = T // BLK                       # 32 time blocks per sequence
KMAX = 1280                         # host impulse-response horizon
NK_CAP = 8

CS = 8                      # sequences per chunk
NCHUNK = SEQ_PER_CORE // CS  # 4
NG = 4                      # channel groups
DG = D // NG                # 16 channels per group
NCOL = CS * TB              # 256 (s, tb) columns per channel

_CACHE = {}
LAST_EXEC_NS = None
_MARSHAL_G = [None]


def _impulse_response(phi, theta):
    """g[k, d] in float64 for k = 0..KMAX-1."""
    g = np.zeros((KMAX, D), dtype=np.float64)
    g[0] = 1.0
    phi64 = phi.astype(np.float64)
    th64 = theta.astype(np.float64)
    for k in range(1, KMAX):
        acc = np.zeros(D, dtype=np.float64)
        if k <= Q:
            acc += th64[:, k - 1]
        for i in range(1, P + 1):
            if k - i >= 0:
                acc += phi64[:, i - 1] * g[k - i]
        g[k] = acc
    return g


def _mu_offset(theta, mu):
    """c[t, d] with c_t + sum_j theta_j c_{t-j} = mu for all t >= 0.

    Adding c to eps makes the single ARMA filter g reproduce the mu term
    exactly (mu must not pass through the MA part, so a constant offset
    would be wrong during the first Q steps)."""
    th64 = theta.astype(np.float64)
    mu64 = mu.astype(np.float64)
    c = np.zeros((T, D), dtype=np.float64)
    for t in range(T):
        acc = mu64.copy()
        for j in range(1, Q + 1):
            if t - j >= 0:
                acc -= th64[:, j - 1] * c[t - j]
        c[t] = acc
    if np.abs(c).max() > 100.0 * max(np.abs(mu64).max(), 1.0):
        raise ValueError("MA polynomial near non-invertible; c_t diverges")
    return c


def _pick_nk(g, dc_scale):
    """Per-channel tap-block count: coherent DC bias + 6-sigma random tail."""
    mu64 = np.abs(np.asarray(dc_scale, dtype=np.float64))
    nk_d = np.zeros(D, dtype=int)
    for d in range(D):
        for nk in range(2, NK_CAP + 1):
            # block j covers taps j*BLK + t - t'; at output position t=0 the
            # guaranteed coverage ends at (nk-1)*BLK.
            tail = g[(nk - 1) * BLK + 1 :, d]
            bias = abs(tail.sum()) * mu64[d]
            sigma = np.sqrt((tail**2).sum())
            if bias + 6 * sigma < 3e-5:
                nk_d[d] = nk
                break
        else:
            raise ValueError("impulse response decays too slowly")
    return nk_d


def _pick_kd(g, dc_scale, nk_d):
    """Per-channel tap horizon K_d: taps with lag > K_d are negligible under
    the same bias+6-sigma criterion as _pick_nk.  Used to trim all-zero
    trailing columns off the j>=1 Toeplitz blocks."""
    mu64 = np.abs(np.asarray(dc_scale, dtype=np.float64))
    kd = np.zeros(D, dtype=int)
    for d in range(D):
        hi = (nk_d[d] - 1) * BLK  # _pick_nk guarantees this horizon works
        for K in range(1, hi + 1):
            tail = g[K + 1 :, d]
            bias = abs(tail.sum()) * mu64[d]
            sigma = np.sqrt((tail**2).sum())
            if bias + 6 * sigma < 3e-5:
                kd[d] = K
                break
        else:
            kd[d] = hi
    return kd


def _pair_ncol(nk_key, kd_key):
    """Stored column count for each (d, j) Toeplitz block: column t of W_j is
    all-zero once its minimum lag j*BLK + t - (BLK-1) exceeds K_d."""
    ncol = {}
    for d in range(D):
        for j in range(nk_key[d]):
            if j == 0:
                ncol[(d, j)] = BLK
            else:
                ncol[(d, j)] = min(BLK, max(1, kd_key[d] - j * BLK + BLK))
    return ncol


def _pair_r0(nk_key, kd_key):
    """First nonzero row of each block: row t' of W_j only carries lags
    j*BLK + t - t' <= K_d for some t >= 0, i.e. t' >= j*BLK - K_d.  Rounded
    down to a multiple of 32 (PE row-group granularity)."""
    r0 = {}
    for d in range(D):
        for j in range(nk_key[d]):
            r = max(0, j * BLK - kd_key[d])
            # engine APs: base partition 64 may span 64 rows; base 32 may
            # only span 32 (can't cross the 64 line), so use 0 or 64 only
            r0[(d, j)] = 64 if r >= 64 else 0
    return r0


def _toeplitz_pair(g, d, j):
    """W[t', t] = g[j*BLK + t - t', d] (zero where k<0), float64."""
    w = np.zeros((BLK, BLK), dtype=np.float64)
    for tp in range(BLK):
        ks = j * BLK - tp
        lo_t = max(0, -ks)
        w[tp, lo_t:] = g[ks + lo_t : ks + BLK, d]
    return w


def _split_waits(nc, limit=1):
    """Walrus in this container rejects instructions carrying more than a
    couple of sync waits.  Move excess waits onto same-engine NOPs placed
    immediately before the offending instruction (program order on the
    engine queue preserves the semantics)."""
    import bass_rust
    import concourse.mybir as mybir

    n_split = 0
    for bb_name, bassbb in list(nc.bb_map.items()):
        bb = bassbb.bb
        insts = list(bb.instructions)
        out = []
        changed = False
        for inst in insts:
            si = inst.sync_info
            if si is not None and len(si.on_wait) > limit:
                waits = list(si.on_wait)
                keep = waits[:limit]
                rest = waits[limit:]
                while rest:
                    chunk, rest = rest[:limit], rest[limit:]
                    nop = bass_rust.InstNoOp(
                        name=f"waitsplit-{n_split}", engine=inst.engine
                    )
                    n_split += 1
                    nop.sync_info = mybir.SyncInfo(on_wait=chunk, on_update=[])
                    nc.register_instruction(nop)
                    out.append(nop)
                inst.sync_info = mybir.SyncInfo(
                    on_wait=keep, on_update=list(si.on_update)
                )
                changed = True
            out.append(inst)
        if changed:
            bb.instructions = out
    return n_split


def _tile_context_cls():
    from concourse.tile import TileContext
    from concourse.vector_clock import ScopedClock, VectorClock

    class TileContextFix(TileContext):
        # This walrus build rejects >2 sync waits on one CTRL instruction
        # ("Too many sync wait commands"), which the stock final drain hits.
        # Split the final-drain waits one-per-NOP on SP; the drain then
        # needs none (program order on SP covers it).
        def _drain_and_barrier(self, tick_clock, wait_clock):
            ticks = list(tick_clock.global_clock)
            for proc, tick in enumerate(ticks):
                if tick <= 0:
                    continue
                nop = self.nc.sync.nop(nofuse=True, hint="drain_wait_split")
                sub = VectorClock(
                    [tick if i == proc else 0 for i in range(len(ticks))]
                )
                wait_clock.add_sem_waits(nop.ins, ScopedClock({None: sub}))
            self.nc.sync.drain()
            self.nc.all_engine_barrier()
            assert self.sems is not None
            popped = self.nc._tile_sem_poison_stack.pop()
            assert popped is self._sem_poison
            self.nc.clear_and_free_semaphores(list(self.sems.allocated().values()))
            self.nc.all_engine_barrier()

    return TileContextFix


def _group_pairs(nk_key):
    """Per group: list of (d, j) pairs, d within the group."""
    groups = []
    for g in range(NG):
        pairs = []
        for d in range(g * DG, (g + 1) * DG):
            for j in range(nk_key[d]):
                pairs.append((d, j))
        groups.append(pairs)
    return groups


def _build_bass(nk_key, kd_key):
    import concourse.bass as bass
    import concourse.mybir as mybir

    TileContextFix = _tile_context_cls()
    f16, f32 = mybir.dt.float16, mybir.dt.float32
    nk_d = list(nk_key)
    groups = _group_pairs(nk_key)
    ncol = _pair_ncol(nk_key, kd_key)
    r0 = _pair_r0(nk_key, kd_key)
    # order blocks by row class (rows = BLK - r0) so each class is one
    # contiguous column range of the weight tile, loadable with a single
    # row-trimmed DMA into partitions [r0, BLK)
    pairs = sorted(
        (p for gg in groups for p in gg), key=lambda p_: (r0[p_], p_)
    )
    poff = {}
    off = 0
    for p_ in pairs:
        poff[p_] = off
        off += ncol[p_]
    wcols = off
    # per row-class column ranges
    classes = []  # (r0, col_start, col_end)
    for p_ in pairs:
        r = r0[p_]
        if classes and classes[-1][0] == r:
            classes[-1][2] = poff[p_] + ncol[p_]
        else:
            classes.append([r, poff[p_], poff[p_] + ncol[p_]])

    nc = bass.Bass()
    # input: SBUF-image [chunk][128][CS*TB*D] fp16, contiguous, (d, s, tb) cols
    e_p = nc.declare_dram_parameter("e", [NCHUNK, BLK, CS * TB * D], f16, isOutput=False)
    wh_ps = []
    for ci, (r, c0, c1) in enumerate(classes):
        wh_ps.append(
            nc.declare_dram_parameter(f"wh{ci}", [BLK - r, c1 - c0], f16, isOutput=False)
        )
    # output: SBUF-image [chunk][group][128][DG*CS*TB] fp16, contiguous
    xout = nc.declare_dram_parameter("x", [NCHUNK, NG, BLK, DG * CS * TB], f16, isOutput=True)

    with TileContextFix(nc) as tc:
        with (
            tc.tile_pool(name="wpool", bufs=1) as wpool,
            tc.tile_pool(name="epool", bufs=3) as epool,
            tc.tile_pool(name="opool", bufs=8) as opool,
            tc.tile_pool(name="pspool", bufs=8, space="PSUM") as pspool,
        ):
            # all Toeplitz weights resident in SBUF for the whole kernel;
            # row-trimmed classes land in the bottom partitions of their
            # column range (their top rows are never read)
            wt = wpool.tile([BLK, wcols], f16, name="wt")
            for ci, (r, c0, c1) in enumerate(classes):
                nc.sync.dma_start(out=wt[r:BLK, c0:c1], in_=wh_ps[ci][:])

            # issue every input DMA up front: SP's SEQ is in-order, so input
            # loads emitted after a chunk's output DMAs would stall behind
            # their semaphore waits and starve the (serialized) DMA engines
            ets = []
            for chunk in range(NCHUNK):
                et = epool.tile([BLK, CS * TB * D], f16, name="et")
                nc.sync.dma_start(out=et[:], in_=e_p[chunk])
                ets.append(et)

            for chunk in range(NCHUNK):
                # et view: [p][d][s][tb] — channel cols contiguous
                e4 = ets[chunk].rearrange("p (d s c) -> p d s c", d=D, s=CS)
                for g in range(NG):
                    ot = opool.tile([BLK, DG * NCOL], f16, name="ot")
                    for dl in range(DG):
                        d = g * DG + dl
                        ps = pspool.tile([BLK, NCOL], f32, name="ps")
                        ps3 = ps.rearrange("p (s c) -> p s c", s=CS)
                        nkd = nk_d[d]
                        for j in range(nkd):
                            o, w = poff[(d, j)], ncol[(d, j)]
                            r = r0[(d, j)]
                            # j>=1 blocks only touch output rows t < w and
                            # contraction rows t' >= r: taps outside the
                            # bottom-left K_d x K_d corner are below the
                            # channel's negligibility horizon
                            nc.tensor.matmul(
                                ps3[0:w, :, j:],
                                wt[r:BLK, o : o + w],
                                e4[r:BLK, d, :, 0 : TB - j],
                                start=(j == 0),
                                stop=(j == nkd - 1),
                            )
                        dst = ot[:, dl * NCOL : (dl + 1) * NCOL]
                        if dl % 2 == 0:
                            nc.scalar.copy(out=dst, in_=ps[:])
                        else:
                            nc.vector.tensor_copy(out=dst, in_=ps[:])
                    nc.sync.dma_start(out=xout[chunk, g], in_=ot[:])
    _split_waits(nc)
    return nc, (pairs, poff, ncol, r0, classes)


def _marshal_inputs(u16, pairs, poff, ncol, r0, classes):
    """Host-side SBUF-image marshaling."""
    # [core, chunk, s, tb, p, d] -> [core, chunk, p, (d, s, tb)]
    a = u16.reshape(NCORES, NCHUNK, CS, TB, BLK, D)
    a = np.ascontiguousarray(a.transpose(0, 1, 4, 5, 2, 3))
    e_dev = a.reshape(NCORES, NCHUNK, BLK, CS * TB * D)

    g64 = _MARSHAL_G[0]
    wmaps = {}
    for ci, (r, c0, c1) in enumerate(classes):
        wmaps[f"wh{ci}"] = np.zeros((BLK - r, c1 - c0), dtype=np.float16)
    for p_ in pairs:
        d, j = p_
        o, w, r = poff[p_], ncol[p_], r0[p_]
        for ci, (cr, c0, c1) in enumerate(classes):
            if cr == r and c0 <= o < c1:
                wmaps[f"wh{ci}"][:, o - c0 : o - c0 + w] = _toeplitz_pair(
                    g64, d, j
                )[r:, :w].astype(np.float16)
                break
        else:
            raise AssertionError("pair not covered by a row class")
    return e_dev, wmaps


def _unmarshal_output(res_list):
    x = np.empty((N, T, D), dtype=np.float32)
    for c in range(NCORES):
        arr = res_list[c]["x"]  # [chunk, g, p, DG*CS*TB] fp16
        a = arr.reshape(NCHUNK, NG, BLK, DG, CS, TB)
        # -> [chunk, s, tb, p, g, dl]
        a = a.transpose(0, 4, 5, 2, 1, 3)
        x[c * SEQ_PER_CORE : (c + 1) * SEQ_PER_CORE] = a.reshape(
            SEQ_PER_CORE, T, D
        ).astype(np.float32)
    return x


def kernel(eps, phi, theta, mu, x0):
    global LAST_EXEC_NS
    eps = np.asarray(eps, dtype=np.float32)
    phi = np.asarray(phi, dtype=np.float32)
    theta = np.asarray(theta, dtype=np.float32)
    mu = np.asarray(mu, dtype=np.float32)
    x0 = np.asarray(x0, dtype=np.float32)

    g = _impulse_response(phi, theta)
    _MARSHAL_G[0] = g
    c = _mu_offset(theta, mu)
    dc = np.abs(c).max(axis=0)
    nk_d = _pick_nk(g, dc)
    nk_key = tuple(int(v) for v in nk_d)
    kd_key = tuple(int(v) for v in _pick_kd(g, dc, nk_d))

    u16 = (eps.astype(np.float64) + c[None, :, :]).astype(np.float16)

    cache_key = (nk_key, kd_key)
    if cache_key not in _CACHE:
        _CACHE[cache_key] = _build_bass(nk_key, kd_key)
    nc, (pairs, poff, ncol, r0, classes) = _CACHE[cache_key]
    e_dev, wmaps = _marshal_inputs(u16, pairs, poff, ncol, r0, classes)

    from concourse.bass_utils import run_bass_kernel_spmd

    core_ids = list(range(NCORES))
    in_maps = [
        {"e": np.ascontiguousarray(e_dev[cid]), **wmaps} for cid in core_ids
    ]
    trace = bool(int(os.environ.get("ARMA_TRACE", "0")))
    res = run_bass_kernel_spmd(nc, in_maps, core_ids, trace=trace)
    LAST_EXEC_NS = res.exec_time_ns

    x = _unmarshal_output(res.results)

    if np.any(x0):
        h0 = np.zeros((T, D), dtype=np.float64)
        phi64 = phi.astype(np.float64)
        hist = [np.zeros(D)] * 3 + [np.ones(D)]
        for t in range(T):
            val = (
                phi64[:, 0] * hist[3]
                + phi64[:, 1] * hist[2]
                + phi64[:, 2] * hist[1]
                + phi64[:, 3] * hist[0]
            )
            h0[t] = val
            hist = hist[1:] + [val]
        x = x + (x0[:, None, :] * h0[None, :, :]).astype(np.float32)
    return x
